# revision 1
# baseline (speedup 1.0000x reference)
"""MinibatchDiscrimination kernel for 8 Trainium2 NeuronCores.

Computes: M = x @ T.reshape(IN, J*K); sq[a,b,j] = ||M[a,j,:]-M[b,j,:]||^2;
feats[a,j] = sum_b exp(-min(sqrt(sq), 10)); out = concat([x, feats], 1).

Sharding: batch rows split across 8 cores (128 rows each). Each core
redundantly computes the full transposed M (MT = T2^T @ x^T) on the PE,
then evaluates its [128, 1024] block of the pairwise matrix per j via the
Gram trick: sq = n_a + n_b - 2*G, where the (-2G + 1 (x) n_b) part comes
from a single K=9 matmul (8 rows of -2*MT_local plus a ones row paired
with an n_b row), and n_a rides for free as the per-partition bias of the
ACT sqrt. Per-core inputs are batch-rotated so every core's own rows land
at columns 0:128, which makes the program identical across cores (SPMD,
no collectives) — the diagonal always lives in the first 128-column block.

The diagonal of sq is forced to +3e38 with one tensor_max against a host
mask (also neutralizing any sqrt(-eps)=NaN risk: min(NaN,10)=10 on DVE),
so the diagonal contributes exactly exp(-10) to the accumulated feats;
a constant (1 - exp(-10)) correction is added at the end.
"""
import numpy as np

B, IN, J, K = 1024, 512, 64, 8
NCORES = 8
ROWS = B // NCORES          # 128 rows per core
JK = J * K                  # 512
NCH = 4                     # jk chunks of 128 rows of MT
JPC = J // NCH              # 16 j's per chunk
CLAMP = 10.0
BIG = 3.0e38
C_DIAG = float(np.exp(np.float32(-10.0)))  # what the diagonal contributes

_PROG = {}


def _build_program():
    import concourse.bacc as bacc
    import concourse.mybir as mybir
    import concourse.tile as tile
    from concourse.tile_rust import add_dep_helper
    from contextlib import ExitStack

    F32 = mybir.dt.float32
    AF = mybir.ActivationFunctionType
    OP = mybir.AluOpType

    nc = bacc.Bacc("TRN2", target_bir_lowering=False, debug=False,
                   num_devices=NCORES)
    xTr = nc.declare_dram_parameter("xTr", [IN, B], F32, isOutput=False)
    T2d = nc.declare_dram_parameter("T2", [IN, JK], F32, isOutput=False)
    BDd = nc.declare_dram_parameter("BD", [128, JPC], F32, isOutput=False)
    DMd = nc.declare_dram_parameter("DMK", [128, 128], F32, isOutput=False)
    ONd = nc.declare_dram_parameter("ONESR", [1, 4 * 128], F32, isOutput=False)
    FEd = nc.declare_dram_parameter("FEATS", [ROWS, J], F32, isOutput=True)

    with tile.TileContext(nc) as tc, ExitStack() as ctx:
        single = ctx.enter_context(tc.tile_pool(name="single", bufs=1))
        mtpool = ctx.enter_context(tc.tile_pool(name="mtpool", bufs=2))
        sqpool = ctx.enter_context(tc.tile_pool(name="sqpool", bufs=2))
        m2tpool = ctx.enter_context(tc.tile_pool(name="m2tpool", bufs=2))
        lhspool = ctx.enter_context(tc.tile_pool(name="lhspool", bufs=2))
        rhspool = ctx.enter_context(tc.tile_pool(name="rhspool", bufs=2))
        spool = ctx.enter_context(tc.tile_pool(name="spool", bufs=JPC))
        epool = ctx.enter_context(tc.tile_pool(name="epool", bufs=2))
        psA = ctx.enter_context(tc.tile_pool(name="psA", bufs=1, space="PSUM"))
        psN = ctx.enter_context(tc.tile_pool(name="psN", bufs=1, space="PSUM"))
        psM = ctx.enter_context(tc.tile_pool(name="psM", bufs=3, space="PSUM"))

        # --- resident inputs -------------------------------------------------
        xt = single.tile([128, 4, B], F32)        # x^T as [i%128, i//128, b]
        nc.sync.dma_start(out=xt, in_=xTr.ap().rearrange("(kt p) b -> p kt b", p=128))
        t2t = single.tile([128, 4, JK], F32)      # T2 as [i%128, i//128, jk]
        nc.sync.dma_start(out=t2t, in_=T2d.ap().rearrange("(kt p) n -> p kt n", p=128))
        bdt = single.tile([128, JPC], F32)
        nc.sync.dma_start(out=bdt, in_=BDd.ap())
        dmt = single.tile([128, 128], F32)
        nc.sync.dma_start(out=dmt, in_=DMd.ap())
        ntt = single.tile([JPC, NCH, B], F32)     # n^T: n[b, ch*16+jj] at [jj, ch, b]
        nloc = single.tile([ROWS, J], F32)        # n for local rows
        feats = single.tile([ROWS, J], F32)

        prev_act = None  # chain ACT ops in program order (table-set batching)

        def act(ins):
            nonlocal prev_act
            if prev_act is not None:
                add_dep_helper(ins.ins, prev_act.ins, reason="act order")
            prev_act = ins

        # DRAM bounce buffers for the partition-restitching DMAs: SBUF-side
        # APs of a DMA must keep the partition dim plain for Tile's dep
        # tracking, so the (jj k) -> k jj reshuffles read from DRAM instead.
        dramp = ctx.enter_context(tc.tile_pool(name="dramp", bufs=1, space="DRAM"))
        mt_d = dramp.tile([JK, B], F32)        # M^T rows (j*8+k), cols b
        m2t_d = dramp.tile([JK, ROWS], F32)    # -2 * MT[:, local]

        for ch in range(NCH):
            # --- MT chunk: rows [128*ch, 128*ch+128) of M^T = T2^T @ x^T ----
            mt = mtpool.tile([128, B], F32, tag="mt")
            for half in range(2):
                pa = psA.tile([128, 512], F32, tag="pa")
                for kt in range(4):
                    nc.tensor.matmul(
                        pa,
                        t2t[:, kt, ch * 128:(ch + 1) * 128],
                        xt[:, kt, half * 512:(half + 1) * 512],
                        start=(kt == 0), stop=(kt == 3),
                    )
                nc.vector.tensor_copy(mt[:, half * 512:(half + 1) * 512], pa)
            nc.gpsimd.dma_start(out=mt_d[ch * 128:(ch + 1) * 128, :], in_=mt)

            # --- n for this chunk's 16 j's ----------------------------------
            sqt = sqpool.tile([128, B], F32, tag="sqt")  # MT^2
            nc.vector.tensor_tensor(out=sqt, in0=mt, in1=mt, op=OP.mult)
            for half in range(2):
                pn = psN.tile([JPC, 512], F32, tag="pn")
                nc.tensor.matmul(
                    pn, bdt, sqt[:, half * 512:(half + 1) * 512],
                    start=True, stop=True,
                )
                nc.vector.tensor_copy(
                    ntt[:, ch, half * 512:(half + 1) * 512], pn)
            # local-row n: contract (MT_local^2) against block-diag ones
            pnl = psA.tile([128, JPC], F32, tag="pa")
            nc.tensor.matmul(pnl, sqt[:, 0:ROWS], bdt, start=True, stop=True)
            nc.vector.tensor_copy(nloc[:, ch * JPC:(ch + 1) * JPC], pnl)

            # --- stitched lhsT for this chunk: [-2*MT_local; ones] ----------
            # two j's are packed per PE round via tile_position row groups
            # (rows 0:9 and 32:41), so lhs/rhs carry both row groups.
            m2t = m2tpool.tile([128, ROWS], F32, tag="m2t")
            nc.vector.tensor_scalar_mul(m2t, mt[:, 0:ROWS], -2.0)
            nc.gpsimd.dma_start(out=m2t_d[ch * 128:(ch + 1) * 128, :], in_=m2t)

            # --- main loop: 16 j's, in 8-j groups, paired (v, v+4) ----------
            s_tiles = []
            for u0 in range(0, JPC, 8):
                base = ch * 128 + u0 * 8
                lhs = lhspool.tile([41, 4 * ROWS], F32, tag="lhs")
                rhs = rhspool.tile([41, 4, B], F32, tag="rhs")
                for hi in range(2):
                    p0 = 32 * hi
                    nc.gpsimd.dma_start(
                        out=lhs[p0:p0 + 8, :].rearrange(
                            "k (jj col) -> k jj col", col=ROWS),
                        in_=m2t_d[base + 32 * hi: base + 32 * hi + 32, :].rearrange(
                            "(jj k) col -> k jj col", k=8),
                    )
                    nc.gpsimd.dma_start(out=lhs[p0 + 8:p0 + 9, :], in_=ONd.ap())
                    nc.gpsimd.dma_start(
                        out=rhs[p0:p0 + 8, :, :],
                        in_=mt_d[base + 32 * hi: base + 32 * hi + 32, :].rearrange(
                            "(u k) b -> k u b", k=8),
                    )
                    nc.gpsimd.dma_start(
                        out=rhs[p0 + 8:p0 + 9, :, :],
                        in_=ntt[u0 + 4 * hi: u0 + 4 * hi + 4, ch, :],
                    )
                for v in range(4):
                    ps_pair = []
                    for hi in range(2):
                        jj = u0 + v + 4 * hi
                        j = ch * JPC + jj
                        p0 = 32 * hi
                        ps = psM.tile([128, B], F32, tag="ps")
                        for half in range(2):
                            nc.tensor.matmul(
                                ps[:, half * 512:(half + 1) * 512],
                                lhs[p0:p0 + 9, v * ROWS:(v + 1) * ROWS],
                                rhs[p0:p0 + 9, v, half * 512:(half + 1) * 512],
                                start=True, stop=True,
                                tile_position=(p0, 0),
                            )
                        ps_pair.append((j, ps))
                    for j, ps in ps_pair:
                        nc.vector.tensor_max(ps[:, 0:ROWS], ps[:, 0:ROWS], dmt)
                        s = spool.tile([128, B], F32, tag="s")
                        act(nc.scalar.activation(s, ps, AF.Sqrt,
                                                 bias=nloc[:, j:j + 1], scale=1.0))
                        s_tiles.append((j, s))
            for j, s in s_tiles:
                nc.vector.tensor_scalar_min(s, s, CLAMP)
            for j, s in s_tiles:
                e = epool.tile([128, B], F32, tag="e")
                act(nc.scalar.activation(e, s, AF.Exp, scale=-1.0,
                                         accum_out=feats[:, j:j + 1]))

        # diagonal contributed exp(-10); reference contributes exp(0) = 1
        nc.vector.tensor_scalar_add(feats, feats, 1.0 - C_DIAG)
        nc.sync.dma_start(out=FEd.ap(), in_=feats)

    nc.finalize()
    return nc


def _get_program():
    if "nc" not in _PROG:
        _PROG["nc"] = _build_program()
    return _PROG["nc"]


def _host_consts():
    bd = np.zeros((128, JPC), dtype=np.float32)
    for p in range(128):
        bd[p, p // 8] = 1.0
    # max(sq, dm): identity off-diag, forces the diagonal to 1e10 so that
    # sqrt stays in ACT's legal range and clamps to 10 deterministically.
    dm = np.full((128, 128), -BIG, dtype=np.float32)
    np.fill_diagonal(dm, 1.0e10)
    ones = np.ones((1, 4 * ROWS), dtype=np.float32)
    return bd, dm, ones


def kernel(x: np.ndarray, T: np.ndarray) -> np.ndarray:
    from concourse.bass_utils import run_bass_kernel_spmd

    x = np.ascontiguousarray(np.asarray(x, dtype=np.float32))
    T = np.ascontiguousarray(np.asarray(T, dtype=np.float32))
    assert x.shape == (B, IN) and T.shape == (IN, J, K)

    nc = _get_program()
    t2 = np.ascontiguousarray(T.reshape(IN, JK))
    bd, dm, ones = _host_consts()

    in_maps = []
    for c in range(NCORES):
        xr = np.roll(x, -c * ROWS, axis=0)            # local rows -> cols 0:128
        in_maps.append({
            "xTr": np.ascontiguousarray(xr.T),
            "T2": t2,
            "BD": bd,
            "DMK": dm,
            "ONESR": ones,
        })

    res = run_bass_kernel_spmd(nc, in_maps, list(range(NCORES)))
    feats = np.concatenate([res.results[c]["FEATS"] for c in range(NCORES)], axis=0)
    return np.concatenate([x, feats.astype(np.float32)], axis=1)



# revision 8
# speedup vs baseline: 1.9935x; 1.9935x over previous
"""MinibatchDiscrimination kernel for 8 Trainium2 NeuronCores.

Computes: M = x @ T.reshape(IN, J*K); sq[a,b,j] = ||M[a,j,:]-M[b,j,:]||^2;
feats[a,j] = sum_b exp(-min(sqrt(sq), 10)); out = concat([x, feats], 1).

Sharding: batch rows split across 8 cores (128 rows each), inputs batch-
rotated per core so local rows land at columns 0:128 (SPMD, no collectives).

Per core, per j the [128, 1024] block of sq is produced by a single 12-row
bf16 matmul (Gram trick): rows 0-7 pair (-2*MT_local) with MT, rows 8-9
pair ones with n_b (split hi/lo so the ~2^14-magnitude n survives bf16:
n = hi + lo with |lo| < 2^-8 |n|), rows 10-11 pair n_a hi/lo with ones,
so PSUM holds the complete sq = n_a + n_b - 2G in fp32.  bf16 products
accumulate exactly in fp32 PSUM, so the Gram cancellation is coherent with
the bf16-rounded M.  A tiny accumulate-matmul of (40*I)^T(40*I) then adds
1600 on the diagonal (sq_diag would otherwise be ~0 +- rounding, a
sqrt-of-negative risk).

sqrt runs on the Vector engine as a float bit hack: bitcast(bits(sq) >> 1)
equals sqrt(sq) * 2^-63.5 up to a factor in [1, 2^(1/12)] (exponent parity
works out; the classic magic-add is only a recentering), so one bitwise
DVE op in place in PSUM plus folding LAM = 2^63.5/center into the Exp
scale gives d to +-3%% - plenty, since exp(-d) only matters for rare d<10.
The Scalar engine computes exp(-LAM*v) in place with accum_out reducing
over b.  The reference's clamp is replaced by the identity
exp(-min(d,10)) ~= exp(-d) + exp(-10) (error <= exp(-10) = 4.5e-5 per
element), so feats = accum + (B-1)*exp(-10) + 1 as one constant add at
the end.  Engine budget: ~76us DVE (bit-sqrt) / ~80us ACT (exp+accum+PSUM
escapes), vs ~145us ACT + ~93us DVE for a sqrt/min/exp formulation.
"""
import numpy as np

B, IN, J, K = 1024, 512, 64, 8
NCORES = 8
ROWS = B // NCORES          # 128 rows per core
JK = J * K                  # 512
NCH = 4                     # jk chunks of 128 rows of MT
JPC = J // NCH              # 16 j's per chunk
LAM = 1.266533333e19        # exp scale: LAM * bitcast(bits(sq) >> 1) ~= sqrt(sq)
DIAG_SQ = 40.0              # (40*I)^T(40*I) puts 1600 on the diagonal
C_CLAMP = float(np.exp(np.float32(-10.0)))

_PROG = {}


def _build_program():
    import concourse.bacc as bacc
    import concourse.mybir as mybir
    import concourse.tile as tile
    from contextlib import ExitStack

    F32 = mybir.dt.float32
    BF16 = mybir.dt.bfloat16
    U32 = mybir.dt.uint32
    AF = mybir.ActivationFunctionType
    OP = mybir.AluOpType

    nc = bacc.Bacc("TRN2", target_bir_lowering=False, debug=False,
                   num_devices=NCORES)
    xTr = nc.declare_dram_parameter("xTr", [IN, B], BF16, isOutput=False)
    T2d = nc.declare_dram_parameter("T2", [IN, JK], BF16, isOutput=False)
    BDd = nc.declare_dram_parameter("BD", [128, JPC], F32, isOutput=False)
    EYd = nc.declare_dram_parameter("EYE", [128, 128], BF16, isOutput=False)
    ONd = nc.declare_dram_parameter("ONESW", [1, 16 * B], BF16, isOutput=False)
    FEd = nc.declare_dram_parameter("FEATS", [ROWS, J], F32, isOutput=True)

    with tile.TileContext(nc) as tc, ExitStack() as ctx:
        single = ctx.enter_context(tc.tile_pool(name="single", bufs=1))
        mtpool = ctx.enter_context(tc.tile_pool(name="mtpool", bufs=2))
        sqpool = ctx.enter_context(tc.tile_pool(name="sqpool", bufs=2))
        ntpool = ctx.enter_context(tc.tile_pool(name="ntpool", bufs=2))
        smpool = ctx.enter_context(tc.tile_pool(name="smpool", bufs=2))
        lhspool = ctx.enter_context(tc.tile_pool(name="lhspool", bufs=2))
        rhspool = ctx.enter_context(tc.tile_pool(name="rhspool", bufs=2))
        psA = ctx.enter_context(tc.tile_pool(name="psA", bufs=1, space="PSUM"))
        psN = ctx.enter_context(tc.tile_pool(name="psN", bufs=1, space="PSUM"))
        psM = ctx.enter_context(tc.tile_pool(name="psM", bufs=3, space="PSUM"))
        dramp = ctx.enter_context(tc.tile_pool(name="dramp", bufs=2,
                                               space="DRAM"))

        # --- resident inputs -------------------------------------------------
        xt = single.tile([128, 4, B], BF16)       # x^T as [i%128, i//128, b]
        for kt in range(4):
            nc.sync.dma_start(
                out=xt[:, kt, :],
                in_=xTr.ap()[kt * 128:(kt + 1) * 128, :])
        t2t = single.tile([128, 4, JK], BF16)     # T2 as [i%128, i//128, jk]
        for kt in range(4):
            nc.sync.dma_start(
                out=t2t[:, kt, :],
                in_=T2d.ap()[kt * 128:(kt + 1) * 128, :])
        bdt = single.tile([128, JPC], F32)
        nc.sync.dma_start(out=bdt, in_=BDd.ap())
        eye = single.tile([128, 128], BF16)
        nc.sync.dma_start(out=eye, in_=EYd.ap())
        feats = single.tile([ROWS, J], F32)

        # per-chunk DRAM bounce images for the stitched gram operands
        chunk_d = []
        for _ in range(2):  # double-buffered by construction (dramp bufs=2)
            chunk_d.append({
                "mt": dramp.tile([128, B], BF16, tag="mt_d", name="mt_d"),
                "m2": dramp.tile([128, ROWS], BF16, tag="m2_d", name="m2_d"),
                "nt": dramp.tile([2 * JPC, B], BF16, tag="nt_d", name="nt_d"),
                "nl": dramp.tile([2 * JPC, ROWS], BF16, tag="nl_d",
                                 name="nl_d"),
            })

        def prep(ch):
            """MT chunk -> DRAM images; n vectors (hi/lo); stitch sources."""
            d = chunk_d[ch % 2]
            # MT rows [128*ch, 128*ch+128) of M^T = T2^T @ x^T  (bf16)
            mt = mtpool.tile([128, B], BF16, tag="mt")
            for half in range(2):
                pa = psA.tile([128, 512], F32, tag="pa")
                for kt in range(4):
                    nc.tensor.matmul(
                        pa,
                        t2t[:, kt, ch * 128:(ch + 1) * 128],
                        xt[:, kt, half * 512:(half + 1) * 512],
                        start=(kt == 0), stop=(kt == 3),
                    )
                # PSUM escape on ACT (Copy, rounds to bf16); DVE is tighter
                nc.scalar.copy(mt[:, half * 512:(half + 1) * 512], pa)
            nc.gpsimd.dma_start(out=d["mt"], in_=mt)
            m2t = smpool.tile([128, ROWS], BF16, tag="m2t")
            nc.vector.tensor_scalar_mul(m2t, mt[:, 0:ROWS], -2.0)
            nc.gpsimd.dma_start(out=d["m2"], in_=m2t)
            # n for this chunk's 16 j's: n^T = BD^T @ (MT*MT), split hi/lo
            sqt = sqpool.tile([128, B], F32, tag="sqt")
            nc.gpsimd.tensor_tensor(out=sqt, in0=mt, in1=mt, op=OP.mult)
            nth = ntpool.tile([JPC, B], BF16, tag="nth")
            ntl = ntpool.tile([JPC, B], BF16, tag="ntl")
            for half in range(2):
                pn = psN.tile([JPC, 512], F32, tag="pn")
                nc.tensor.matmul(
                    pn, bdt, sqt[:, half * 512:(half + 1) * 512],
                    start=True, stop=True)
                sl = slice(half * 512, (half + 1) * 512)
                nc.scalar.copy(nth[:, sl], pn)               # hi = bf16(n)
                nc.vector.tensor_tensor(                     # lo = n - hi
                    out=ntl[:, sl], in0=pn, in1=nth[:, sl], op=OP.subtract)
            nc.gpsimd.dma_start(out=d["nt"][0:JPC, :], in_=nth)
            nc.gpsimd.dma_start(out=d["nt"][JPC:2 * JPC, :], in_=ntl)
            pl = psN.tile([JPC, ROWS], F32, tag="pn")
            nc.tensor.matmul(pl, bdt, sqt[:, 0:ROWS], start=True, stop=True)
            nlh = smpool.tile([JPC, ROWS], BF16, tag="nlh")
            nll = smpool.tile([JPC, ROWS], BF16, tag="nll")
            nc.scalar.copy(nlh, pl)
            nc.vector.tensor_tensor(out=nll, in0=pl, in1=nlh, op=OP.subtract)
            nc.gpsimd.dma_start(out=d["nl"][0:JPC, :], in_=nlh)
            nc.gpsimd.dma_start(out=d["nl"][JPC:2 * JPC, :], in_=nll)
            # stitched operands: 2 groups of 8 j's
            # lhsT rows: 0-7 = -2*MT_local, 8-9 = ones, 10-11 = n_a hi/lo
            # rhs  rows: 0-7 = MT,          8-9 = n_b hi/lo, 10-11 = ones
            stitched = []
            for g in range(2):
                r0 = g * 64
                lhs = lhspool.tile([12, 8, ROWS], BF16, tag="lhs")
                rhs = rhspool.tile([12, 8, B], BF16, tag="rhs")
                nc.gpsimd.dma_start(
                    out=lhs[0:8, :, :],
                    in_=d["m2"][r0:r0 + 64, :].rearrange(
                        "(jj k) c -> k jj c", k=8))
                nc.gpsimd.dma_start(
                    out=lhs[8:10, :, :].rearrange("p jj c -> p (jj c)"),
                    in_=ONd.ap()[:, 0:2 * 8 * ROWS].rearrange(
                        "o (p n) -> (o p) n", p=2))
                nc.gpsimd.dma_start(
                    out=lhs[10:12, :, :].rearrange("p jj c -> p (jj c)"),
                    in_=d["nl"].rearrange("(h jj) c -> h jj c", h=2)[
                        :, g * 8:(g + 1) * 8, :].rearrange(
                        "h jj c -> h (jj c)"))
                nc.gpsimd.dma_start(
                    out=rhs[0:8, :, :],
                    in_=d["mt"][r0:r0 + 64, :].rearrange(
                        "(jj k) b -> k jj b", k=8))
                nc.gpsimd.dma_start(
                    out=rhs[8:10, :, :].rearrange("p jj b -> p (jj b)"),
                    in_=d["nt"].rearrange("(h jj) b -> h jj b", h=2)[
                        :, g * 8:(g + 1) * 8, :].rearrange(
                        "h jj b -> h (jj b)"))
                nc.gpsimd.dma_start(
                    out=rhs[10:12, :, :].rearrange("p jj b -> p (jj b)"),
                    in_=ONd.ap().rearrange("o (p n) -> (o p) n", p=2))
                stitched.append((lhs, rhs))
            return stitched

        stitched = prep(0)
        for ch in range(NCH):
            nxt = prep(ch + 1) if ch + 1 < NCH else None
            for g in range(2):
                lhs, rhs = stitched[g]
                for v in range(8):
                    j = ch * JPC + g * 8 + v
                    ps = psM.tile([128, B], F32, tag="ps")
                    nc.tensor.matmul(
                        ps[:, 0:512], lhs[:, v, :], rhs[:, v, 0:512],
                        start=True, stop=False, skip_group_check=True)
                    nc.tensor.matmul(
                        ps[:, 0:128], eye, eye,
                        start=False, stop=True, skip_group_check=True)
                    nc.tensor.matmul(
                        ps[:, 512:1024], lhs[:, v, :], rhs[:, v, 512:1024],
                        start=True, stop=True)
                    # LAM * bitcast(bits(sq) >> 1) ~= sqrt(sq)  (+-3%)
                    psu = ps.bitcast(U32)
                    nc.vector.tensor_scalar(
                        out=psu, in0=psu, scalar1=1, scalar2=None,
                        op0=OP.logical_shift_right)
                    # e = exp(-sqrt(sq)), accumulated over b into feats[:, j]
                    nc.scalar.activation(
                        ps, ps, AF.Exp, scale=-LAM,
                        accum_out=feats[:, j:j + 1])
            stitched = nxt

        # off-diag clamp floor (B-1)*exp(-10); diagonal contributes exactly 1
        nc.vector.tensor_scalar_add(feats, feats, 1.0 + (B - 1) * C_CLAMP)
        nc.sync.dma_start(out=FEd.ap(), in_=feats)

    nc.finalize()
    return nc


def _get_program():
    if "nc" not in _PROG:
        _PROG["nc"] = _build_program()
    return _PROG["nc"]


def _bf16(a):
    import ml_dtypes
    return np.asarray(a, dtype=ml_dtypes.bfloat16)


def _host_consts():
    bd = np.zeros((128, JPC), dtype=np.float32)
    for p in range(128):
        bd[p, p // 8] = 1.0
    eye = _bf16(np.eye(128, dtype=np.float32) * DIAG_SQ)
    ones = _bf16(np.ones((1, 16 * B), dtype=np.float32))
    return bd, eye, ones


def kernel(x: np.ndarray, T: np.ndarray) -> np.ndarray:
    from concourse.bass_utils import run_bass_kernel_spmd

    x = np.ascontiguousarray(np.asarray(x, dtype=np.float32))
    T = np.ascontiguousarray(np.asarray(T, dtype=np.float32))
    assert x.shape == (B, IN) and T.shape == (IN, J, K)

    nc = _get_program()
    t2 = _bf16(np.ascontiguousarray(T.reshape(IN, JK)))
    bd, eye, ones = _host_consts()

    in_maps = []
    for c in range(NCORES):
        xr = np.roll(x, -c * ROWS, axis=0)            # local rows -> cols 0:128
        in_maps.append({
            "xTr": _bf16(np.ascontiguousarray(xr.T)),
            "T2": t2,
            "BD": bd,
            "EYE": eye,
            "ONESW": ones,
        })

    res = run_bass_kernel_spmd(nc, in_maps, list(range(NCORES)))
    feats = np.concatenate([res.results[c]["FEATS"] for c in range(NCORES)], axis=0)
    return np.concatenate([x, feats.astype(np.float32)], axis=1)


# revision 17
# speedup vs baseline: 2.4690x; 1.2385x over previous
"""MinibatchDiscrimination kernel for 8 Trainium2 NeuronCores.

Computes: M = x @ T.reshape(IN, J*K); sq[a,b,j] = ||M[a,j,:]-M[b,j,:]||^2;
feats[a,j] = sum_b exp(-min(sqrt(sq), 10)); out = concat([x, feats], 1).

Sharding: batch rows split across 8 cores (128 rows each), inputs batch-
rotated per core so local rows land at columns 0:128 (SPMD, no collectives).

Per core, per j the [128, 1024] block of sq = n_a + n_b - 2G builds up in
fp32 PSUM from three accumulating matmuls:
  1. an 8-row bf16 Gram matmul pairing -2*MT_local with MT (bf16 products
     accumulate exactly in fp32, so the cancellation is coherent with the
     bf16-rounded M); operands come k-major from a small DRAM restitch.
  2. a 17-row fp32r matmul: 16 constant one-hot rows select row jj of the
     resident n tile (adding n_b), and row 16 pairs n_a with ones - fp32r
     keeps ~12 mantissa bits, so the ~2^14-magnitude n rows survive where
     bf16 would lose the cancellation; operands are direct SBUF slices.
  3. a bf16 (40*I)^T(40*I) matmul adding 1600 on the diagonal (sq_diag
     would otherwise be ~0 +- rounding, a sqrt-of-negative risk).

sqrt runs on the Vector engine as a float bit hack: bitcast(bits(sq) >> 1)
equals sqrt(sq) * 2^-63.5 up to a factor in [1, 2^(1/12)] (exponent parity
works out; the classic magic-add is only a recentering), so one bitwise
DVE op in place in PSUM plus folding LAM = 2^63.5/center into the Exp
scale gives d to +-3% - plenty, since exp(-d) only matters for rare d<10.
The Scalar engine computes exp(-LAM*v) in place with accum_out reducing
over b.  The reference's clamp is replaced by the identity
exp(-min(d,10)) ~= exp(-d) + exp(-10) (error <= exp(-10) = 4.5e-5 per
element), so feats = accum + (B-1)*exp(-10) + 1 as one constant add at
the end.

Scheduling: chunks 0/1 prep in the prologue while input DMAs land; chunk
ch+2's prep is emitted a few j's into chunk ch's loop so engine queues
interleave prep with the steady j pipeline.  All DMAs issue from SP/ACT
HWDGE queues (SWDGE descriptor generation would occupy the GPSIMD engine
~1us per DMA); the single HWDGE generator costs ~0.6us per DMA, so DMAs
are consolidated.  Engine budget/core: ~81us DVE (bit-sqrt), ~81us ACT
(exp+accum + PSUM escapes), ~72us PE, ~8us GPSIMD.
"""
import numpy as np

B, IN, J, K = 1024, 512, 64, 8
NCORES = 8
ROWS = B // NCORES          # 128 rows per core
JK = J * K                  # 512
NCH = 4                     # jk chunks of 128 rows of MT
JPC = J // NCH              # 16 j's per chunk
LAM = 1.266533333e19        # exp scale: LAM * bitcast(bits(sq) >> 1) ~= sqrt(sq)
DIAG_SQ = 40.0              # (40*I)^T(40*I) puts 1600 on the diagonal
C_CLAMP = float(np.exp(np.float32(-10.0)))

_PROG = {}


def _build_program():
    import concourse.bacc as bacc
    import concourse.mybir as mybir
    import concourse.tile as tile
    from contextlib import ExitStack

    F32 = mybir.dt.float32
    F32R = mybir.dt.float32r
    BF16 = mybir.dt.bfloat16
    U32 = mybir.dt.uint32
    AF = mybir.ActivationFunctionType
    OP = mybir.AluOpType

    nc = bacc.Bacc("TRN2", target_bir_lowering=False, debug=False,
                   num_devices=NCORES)
    xTr = nc.declare_dram_parameter("xTr", [IN, B], BF16, isOutput=False)
    T2d = nc.declare_dram_parameter("T2", [IN, JK], BF16, isOutput=False)
    BDd = nc.declare_dram_parameter("BD", [128, JPC], F32, isOutput=False)
    EYd = nc.declare_dram_parameter("EYE", [128, 128], BF16, isOutput=False)
    ONd = nc.declare_dram_parameter("ONESW", [1, B], F32R, isOutput=False)
    OHd = nc.declare_dram_parameter("OH", [JPC, JPC * ROWS], F32R,
                                    isOutput=False)
    FEd = nc.declare_dram_parameter("FEATS", [ROWS, J], F32, isOutput=True)

    with tile.TileContext(nc) as tc, ExitStack() as ctx:
        single = ctx.enter_context(tc.tile_pool(name="single", bufs=1))
        mtpool = ctx.enter_context(tc.tile_pool(name="mtpool", bufs=2))
        sqpool = ctx.enter_context(tc.tile_pool(name="sqpool", bufs=2))
        smpool = ctx.enter_context(tc.tile_pool(name="smpool", bufs=2))
        lhspool = ctx.enter_context(tc.tile_pool(name="lhspool", bufs=3))
        rhspool = ctx.enter_context(tc.tile_pool(name="rhspool", bufs=3))
        nbpool = ctx.enter_context(tc.tile_pool(name="nbpool", bufs=3))
        nhpool = ctx.enter_context(tc.tile_pool(name="nhpool", bufs=3))
        psA = ctx.enter_context(tc.tile_pool(name="psA", bufs=1, space="PSUM"))
        psN = ctx.enter_context(tc.tile_pool(name="psN", bufs=1, space="PSUM"))
        psM = ctx.enter_context(tc.tile_pool(name="psM", bufs=3, space="PSUM"))
        dramp = ctx.enter_context(tc.tile_pool(name="dramp", bufs=2,
                                               space="DRAM"))

        # --- resident inputs ------------------------------------------------
        t2t = single.tile([128, 4, JK], BF16)     # T2 as [i%128, i//128, jk]
        nc.sync.dma_start(
            out=t2t, in_=T2d.ap().rearrange("(kt p) n -> p kt n", p=128))
        bdt = single.tile([128, JPC], F32)
        nc.sync.dma_start(out=bdt, in_=BDd.ap())
        eye = single.tile([128, 128], BF16)
        nc.sync.dma_start(out=eye, in_=EYd.ap())
        xt = single.tile([128, 4, B], BF16)       # x^T as [i%128, i//128, b]
        for kt in range(4):   # per-kt so chunk 0's MT can start early
            nc.scalar.dma_start(
                out=xt[:, kt, 0:512],
                in_=xTr.ap()[kt * 128:(kt + 1) * 128, 0:512])
        nc.sync.dma_start(
            out=xt[:, :, 512:1024],
            in_=xTr.ap().rearrange("(kt p) b -> p kt b", p=128)[
                :, :, 512:1024])
        feats = single.tile([ROWS, J], F32)

        # per-chunk DRAM bounce images for the k-major gram operand restitch
        chunk_d = []
        for _ in range(2):  # double-buffered by construction (dramp bufs=2)
            chunk_d.append({
                "mt": dramp.tile([128, B], BF16, tag="mt_d", name="mt_d"),
                "m2": dramp.tile([128, ROWS], BF16, tag="m2_d", name="m2_d"),
            })

        def prep_a(ch):
            """MT chunk -> SBUF + DRAM images of MT and -2*MT_local."""
            d = chunk_d[ch % 2]
            mt = mtpool.tile([128, B], BF16, tag="mt")
            for half in range(2):
                pa = psA.tile([128, 512], F32, tag="pa")
                for kt in range(4):
                    nc.tensor.matmul(
                        pa,
                        t2t[:, kt, ch * 128:(ch + 1) * 128],
                        xt[:, kt, half * 512:(half + 1) * 512],
                        start=(kt == 0), stop=(kt == 3),
                    )
                # PSUM escape (Copy rounds to bf16), split ACT/DVE for balance
                if half == 0:
                    nc.scalar.copy(mt[:, 0:512], pa)
                else:
                    nc.vector.tensor_copy(mt[:, 512:1024], pa)
            nc.sync.dma_start(out=d["mt"], in_=mt)
            m2t = smpool.tile([128, ROWS], BF16, tag="m2t")
            nc.gpsimd.tensor_scalar_mul(m2t, mt[:, 0:ROWS], -2.0)
            nc.sync.dma_start(out=d["m2"], in_=m2t)
            return mt

        def prep_b(ch, mt):
            """n rows for the chunk: fp32r tiles feeding the bias matmul.
            nlhs partitions: 0 = ones, 1 = n_a;  nrhs: 0 = n_b, 1 = ones.
            """
            sqt = sqpool.tile([128, B], F32, tag="sqt")
            sq_eng = nc.vector if ch == 0 else nc.gpsimd
            sq_eng.tensor_tensor(out=sqt, in0=mt, in1=mt, op=OP.mult)
            # nrh partitions 0:16 = n rows, partition 16 = ones
            nrh = nhpool.tile([JPC + 1, B], F32R, tag="nrh")
            for half in range(2):
                pn = psN.tile([JPC, 512], F32, tag="pn")
                nc.tensor.matmul(
                    pn, bdt, sqt[:, half * 512:(half + 1) * 512],
                    start=True, stop=True)
                sl = slice(half * 512, (half + 1) * 512)
                if half == 0:
                    nc.scalar.copy(nrh[0:JPC, sl], pn)  # rounds to fp32r
                else:
                    nc.vector.tensor_copy(nrh[0:JPC, sl], pn)
            nc.sync.dma_start(out=nrh[JPC:JPC + 1, :], in_=ONd.ap())
            # nlh rows 0:16 = one-hot jj selectors, row 16 = n_a
            nlh = nbpool.tile([JPC + 1, JPC, ROWS], F32R, tag="nlh")
            nc.sync.dma_start(
                out=nlh[0:JPC, :, :].rearrange("p jj c -> p (jj c)"),
                in_=OHd.ap())
            nc.sync.dma_start(out=nlh[JPC:JPC + 1, :, :],
                              in_=nrh[0:JPC, 0:ROWS])
            return nlh, nrh

        def prep_c(ch):
            """Stitch the chunk's k-major gram operands from DRAM."""
            d = chunk_d[ch % 2]
            lhs = lhspool.tile([8, JPC, ROWS], BF16, tag="lhs")
            rhs = rhspool.tile([8, JPC, B], BF16, tag="rhs")
            nc.sync.dma_start(
                out=lhs, in_=d["m2"].rearrange("(jj k) c -> k jj c", k=8))
            nc.sync.dma_start(
                out=rhs, in_=d["mt"].rearrange("(jj k) b -> k jj b", k=8))
            return lhs, rhs

        def jwork(ch, gram, nbias, jj):
            lhs, rhs = gram
            nlh, nrh = nbias
            j = ch * JPC + jj
            ps = psM.tile([128, B], F32, tag="ps")
            nc.tensor.matmul(
                ps[:, 0:512], lhs[:, jj, :], rhs[:, jj, 0:512],
                start=True, stop=False, skip_group_check=True)
            nc.tensor.matmul(
                ps[:, 0:512], nlh[:, jj, :], nrh[:, 0:512],
                start=False, stop=False, skip_group_check=True)
            nc.tensor.matmul(
                ps[:, 0:128], eye, eye,
                start=False, stop=True, skip_group_check=True)
            nc.tensor.matmul(
                ps[:, 512:1024], lhs[:, jj, :], rhs[:, jj, 512:1024],
                start=True, stop=False, skip_group_check=True)
            nc.tensor.matmul(
                ps[:, 512:1024], nlh[:, jj, :], nrh[:, 512:1024],
                start=False, stop=True, skip_group_check=True)
            # LAM * bitcast(bits(sq) >> 1) ~= sqrt(sq)  (+-3%)
            psu = ps.bitcast(U32)
            nc.vector.tensor_scalar(
                out=psu, in0=psu, scalar1=1, scalar2=None,
                op0=OP.logical_shift_right)
            # e = exp(-sqrt(sq)), accumulated over b into feats[:, j]
            nc.scalar.activation(
                ps, ps, AF.Exp, scale=-LAM,
                accum_out=feats[:, j:j + 1])

        # prologue: chunks 0 and 1 fully prepped while input DMAs land;
        # steady state interleaves chunk ch+2's prep into chunk ch's loop.
        mt0 = prep_a(0)
        nb = prep_b(0, mt0)
        gram = prep_c(0)
        mt1 = prep_a(1)
        nb_nxt = prep_b(1, mt1)
        gram_nxt = prep_c(1)
        for ch in range(NCH):
            nn_mt = None
            j_a, j_b, j_c = (8, 10, 12) if ch == 0 else (0, 2, 4)
            for jj in range(JPC):
                jwork(ch, gram, nb, jj)
                if ch + 2 < NCH:
                    if jj == j_a:
                        nn_mt = prep_a(ch + 2)
                    elif jj == j_b:
                        nb_nn = prep_b(ch + 2, nn_mt)
                    elif jj == j_c:
                        gram_nn = prep_c(ch + 2)
            if ch + 1 < NCH:
                gram, nb = gram_nxt, nb_nxt
            if ch + 2 < NCH:
                gram_nxt, nb_nxt = gram_nn, nb_nn

        # off-diag clamp floor (B-1)*exp(-10); diagonal contributes exactly 1
        nc.gpsimd.tensor_scalar_add(feats, feats, 1.0 + (B - 1) * C_CLAMP)
        nc.sync.dma_start(out=FEd.ap(), in_=feats)

    nc.finalize()
    return nc


def _get_program():
    if "nc" not in _PROG:
        _PROG["nc"] = _build_program()
    return _PROG["nc"]


def _bf16(a):
    import ml_dtypes
    return np.asarray(a, dtype=ml_dtypes.bfloat16)


def _host_consts():
    bd = np.zeros((128, JPC), dtype=np.float32)
    for p in range(128):
        bd[p, p // 8] = 1.0
    eye = _bf16(np.eye(128, dtype=np.float32) * DIAG_SQ)
    ones = np.ones((1, B), dtype=np.float32)
    oh = np.zeros((JPC, JPC, ROWS), dtype=np.float32)
    for jj in range(JPC):
        oh[jj, jj, :] = 1.0
    return bd, eye, ones, oh.reshape(JPC, JPC * ROWS)


def kernel(x: np.ndarray, T: np.ndarray) -> np.ndarray:
    from concourse.bass_utils import run_bass_kernel_spmd

    x = np.ascontiguousarray(np.asarray(x, dtype=np.float32))
    T = np.ascontiguousarray(np.asarray(T, dtype=np.float32))
    assert x.shape == (B, IN) and T.shape == (IN, J, K)

    nc = _get_program()
    t2 = _bf16(np.ascontiguousarray(T.reshape(IN, JK)))
    bd, eye, ones, oh = _host_consts()

    in_maps = []
    for c in range(NCORES):
        xr = np.roll(x, -c * ROWS, axis=0)            # local rows -> cols 0:128
        in_maps.append({
            "xTr": _bf16(np.ascontiguousarray(xr.T)),
            "T2": t2,
            "BD": bd,
            "EYE": eye,
            "ONESW": ones,
            "OH": oh,
        })

    res = run_bass_kernel_spmd(nc, in_maps, list(range(NCORES)))
    feats = np.concatenate([res.results[c]["FEATS"] for c in range(NCORES)], axis=0)
    return np.concatenate([x, feats.astype(np.float32)], axis=1)


# revision 39
# speedup vs baseline: 2.7351x; 1.1078x over previous
"""MinibatchDiscrimination kernel for 8 Trainium2 NeuronCores.

Computes: M = x @ T.reshape(IN, J*K); sq[a,b,j] = ||M[a,j,:]-M[b,j,:]||^2;
feats[a,j] = sum_b exp(-min(sqrt(sq), 10)); out = concat([x, feats], 1).

Sharding: batch rows split across 8 cores (128 rows each), inputs batch-
rotated per core so local rows land at columns 0:128 (SPMD, no collectives).

Per core, per j the [128, 1024] block of sq = n_a + n_b - 2G builds up in
fp32 PSUM from three accumulating matmuls:
  1. an 8-row bf16 Gram matmul pairing -2*MT_local with MT (bf16 products
     accumulate exactly in fp32, so the cancellation is coherent with the
     bf16-rounded M); operands come k-major from a small DRAM restitch.
  2. a 17-row fp32r matmul: 16 constant one-hot rows select row jj of the
     resident n tile (adding n_b), and row 16 pairs n_a with ones - fp32r
     keeps ~12 mantissa bits, so the ~2^14-magnitude n rows survive where
     bf16 would lose the cancellation; operands are direct SBUF slices.
  3. a bf16 (40*I)^T(40*I) matmul adding 1600 on the diagonal (sq_diag
     would otherwise be ~0 +- rounding, a sqrt-of-negative risk).

sqrt runs on the Vector engine as a float bit hack: bitcast(bits(sq) >> 1)
equals sqrt(sq) * 2^-63.5 up to a factor in [1, 2^(1/12)] (exponent parity
works out; the classic magic-add is only a recentering), so one bitwise
DVE op in place in PSUM plus folding LAM = 2^63.5/center into the Exp
scale gives d to +-3% - plenty, since exp(-d) only matters for rare d<10.
The Scalar engine computes exp(-LAM*v) in place with accum_out reducing
over b.  The reference's clamp is replaced by the identity
exp(-min(d,10)) ~= exp(-d) + exp(-10) (error <= exp(-10) = 4.5e-5 per
element), so feats = accum + (B-1)*exp(-10) + 1 as one constant add at
the end.

Scheduling: chunks 0/1 prep in the prologue while input DMAs land; chunk
ch+2's prep is emitted a few j's into chunk ch's loop so engine queues
interleave prep with the steady j pipeline.  All DMAs issue from SP/ACT
HWDGE queues (SWDGE descriptor generation would occupy the GPSIMD engine
~1us per DMA); the single HWDGE generator costs ~0.6us per DMA, so DMAs
are consolidated.  Engine budget/core: ~81us DVE (bit-sqrt), ~81us ACT
(exp+accum + PSUM escapes), ~72us PE, ~8us GPSIMD.
"""
import numpy as np

B, IN, J, K = 1024, 512, 64, 8
NCORES = 8
ROWS = B // NCORES          # 128 rows per core
JK = J * K                  # 512
NCH = 4                     # jk chunks of 128 rows of MT
JPC = J // NCH              # 16 j's per chunk
LAM = 1.266533333e19        # exp scale: LAM * bitcast(bits(sq) >> 1) ~= sqrt(sq)
DIAG_SQ = 40.0              # (40*I)^T(40*I) puts 1600 on the diagonal
C_CLAMP = float(np.exp(np.float32(-10.0)))

_PROG = {}


def _build_program():
    import concourse.bacc as bacc
    import concourse.mybir as mybir
    import concourse.tile as tile
    from contextlib import ExitStack

    F32 = mybir.dt.float32
    F32R = mybir.dt.float32r
    BF16 = mybir.dt.bfloat16
    U32 = mybir.dt.uint32
    AF = mybir.ActivationFunctionType
    OP = mybir.AluOpType

    nc = bacc.Bacc("TRN2", target_bir_lowering=False, debug=False,
                   num_devices=NCORES)
    xTr = nc.declare_dram_parameter("xTr", [IN, B], BF16, isOutput=False)
    T2d = nc.declare_dram_parameter("T2", [IN, JK], BF16, isOutput=False)
    BDd = nc.declare_dram_parameter("BD", [128, JPC], F32, isOutput=False)
    EYd = nc.declare_dram_parameter("EYE", [128, 128], BF16, isOutput=False)
    ONd = nc.declare_dram_parameter("ONESW", [1, B], F32R, isOutput=False)
    OHd = nc.declare_dram_parameter("OH", [JPC, JPC * ROWS], F32R,
                                    isOutput=False)
    FEd = nc.declare_dram_parameter("FEATS", [ROWS, J], F32, isOutput=True)

    with tile.TileContext(nc) as tc, ExitStack() as ctx:
        single = ctx.enter_context(tc.tile_pool(name="single", bufs=1))
        mtpool = ctx.enter_context(tc.tile_pool(name="mtpool", bufs=3))
        sqpool = ctx.enter_context(tc.tile_pool(name="sqpool", bufs=3))
        smpool = ctx.enter_context(tc.tile_pool(name="smpool", bufs=3))
        lhspool = ctx.enter_context(tc.tile_pool(name="lhspool", bufs=3))
        rhspool = ctx.enter_context(tc.tile_pool(name="rhspool", bufs=3))
        nbpool = ctx.enter_context(tc.tile_pool(name="nbpool", bufs=3))
        nhpool = ctx.enter_context(tc.tile_pool(name="nhpool", bufs=3))
        psM = ctx.enter_context(tc.tile_pool(name="psM", bufs=4, space="PSUM"))
        dramp = ctx.enter_context(tc.tile_pool(name="dramp", bufs=2,
                                               space="DRAM"))

        # --- resident inputs ------------------------------------------------
        t2t = single.tile([128, 4, JK], BF16)     # T2 as [i%128, i//128, jk]
        nc.sync.dma_start(
            out=t2t, in_=T2d.ap().rearrange("(kt p) n -> p kt n", p=128))
        bdt = single.tile([128, JPC], F32)
        nc.sync.dma_start(out=bdt, in_=BDd.ap())
        eye = single.tile([128, 128], BF16)
        nc.sync.dma_start(out=eye, in_=EYd.ap())
        xt = single.tile([128, 4, B], BF16)       # x^T as [i%128, i//128, b]
        for half in range(2):
            eng = nc.scalar if half == 0 else nc.sync
            eng.dma_start(
                out=xt[:, :, half * 512:(half + 1) * 512],
                in_=xTr.ap().rearrange("(kt p) b -> p kt b", p=128)[
                    :, :, half * 512:(half + 1) * 512])
        feats = single.tile([ROWS, J], F32)

        # spin the Tensor engine on junk matmuls while input DMAs land so the
        # first real matmuls run at full p-state (cold PE is ~3.7x slower)
        warm = single.tile([1, 64], BF16)
        nc.gpsimd.memset(warm, 1.0)
        psw = psM.tile([128, B], F32, tag="ps", name="psw")
        for _ in range(120):
            nc.tensor.matmul(psw[0:64, 0:64], warm, warm,
                             start=True, stop=True, skip_group_check=True)

        # per-chunk DRAM bounce images for the k-major gram operand restitch
        chunk_d = []
        for _ in range(2):  # double-buffered by construction (dramp bufs=2)
            chunk_d.append({
                "mt": dramp.tile([128, B], BF16, tag="mt_d", name="mt_d"),
                "m2": dramp.tile([128, ROWS], BF16, tag="m2_d", name="m2_d"),
            })

        def prep_a_half(ch, st, half):
            """MT chunk half -> SBUF (+DRAM images once complete)."""
            d = chunk_d[ch % 2]
            if half == 0:
                st["mt"] = mtpool.tile([128, B], BF16, tag="mt", name="mt")
                st["m2t"] = smpool.tile([128, ROWS], BF16, tag="m2t", name="m2t")
                st["pa"] = psM.tile([128, B], F32, tag="ps", name="pa")
            mt, m2t, pa = st["mt"], st["m2t"], st["pa"]
            sl = slice(half * 512, (half + 1) * 512)
            for kt in range(4):
                nc.tensor.matmul(
                    pa[:, sl],
                    t2t[:, kt, ch * 128:(ch + 1) * 128],
                    xt[:, kt, half * 512:(half + 1) * 512],
                    start=(kt == 0), stop=(kt == 3),
                    skip_group_check=True,
                )
            # PSUM escape (Copy rounds to bf16), split ACT/DVE for balance
            if half == 0:
                nc.scalar.copy(mt[:, 0:512], pa[:, 0:512])
                nc.gpsimd.tensor_scalar_mul(m2t, mt[:, 0:ROWS], -2.0)
            else:
                nc.vector.tensor_copy(mt[:, 512:1024], pa[:, 512:1024])
                nc.sync.dma_start(out=d["mt"], in_=mt)
                nc.sync.dma_start(out=d["m2"], in_=m2t)

        def prep_a(ch):
            st = {}
            prep_a_half(ch, st, 0)
            prep_a_half(ch, st, 1)
            return st

        def prep_b_half(ch, st, half):
            """Half of the chunk's n rows (square, reduce, fp32r escape)."""
            mt = st["mt"]
            if half == 0:
                st["sqt"] = sqpool.tile([128, B], F32, tag="sqt", name="sqt")
                st["nrh"] = nhpool.tile([JPC + 1, B], F32R, tag="nrh", name="nrh")
                st["pq"] = psM.tile([128, B], F32, tag="ps", name="pq")
            sqt, nrh, pq = st["sqt"], st["nrh"], st["pq"]
            sq_eng = nc.vector if ch == 0 else nc.gpsimd
            sl = slice(half * 512, (half + 1) * 512)
            sq_eng.tensor_tensor(out=sqt[:, sl], in0=mt[:, sl],
                                 in1=mt[:, sl], op=OP.mult)
            nc.tensor.matmul(pq[0:JPC, sl], bdt, sqt[:, sl],
                             start=True, stop=True, skip_group_check=True)
            if half == 0:
                nc.scalar.copy(nrh[0:JPC, sl], pq[0:JPC, sl])
            else:
                nc.vector.tensor_copy(nrh[0:JPC, sl], pq[0:JPC, sl])

        def _mk_nb(st):
            """Finish the bias operands: ones row and the nlh lhsT tile."""
            nrh = st["nrh"]
            # nlh rows 0:16 = one-hot jj selectors, row 16 = n_a
            nlh = nbpool.tile([JPC + 1, JPC, ROWS], F32R, tag="nlh")
            nc.sync.dma_start(
                out=nlh[0:JPC, :, :].rearrange("p jj c -> p (jj c)"),
                in_=OHd.ap())
            nc.sync.dma_start(out=nlh[JPC:JPC + 1, :, :],
                              in_=nrh[0:JPC, 0:ROWS])
            return nlh, nrh

        def prep_b(ch, st):
            """n rows for the chunk: fp32r tiles feeding the bias matmul."""
            prep_b_half(ch, st, 0)
            prep_b_half(ch, st, 1)
            nc.sync.dma_start(out=st["nrh"][JPC:JPC + 1, :], in_=ONd.ap())
            return _mk_nb(st)

        def prep_c(ch):
            """Stitch the chunk's k-major gram operands from DRAM."""
            d = chunk_d[ch % 2]
            lhs = lhspool.tile([8, JPC, ROWS], BF16, tag="lhs")
            rhs = rhspool.tile([8, JPC, B], BF16, tag="rhs")
            nc.sync.dma_start(
                out=lhs, in_=d["m2"].rearrange("(jj k) c -> k jj c", k=8))
            nc.sync.dma_start(
                out=rhs, in_=d["mt"].rearrange("(jj k) b -> k jj b", k=8))
            return lhs, rhs

        def jwork(ch, gram, nbias, jj, direct=None):
            lhs, rhs = gram
            nlh, nrh = nbias
            j = ch * JPC + jj
            if direct is not None:
                # 32-aligned j: gram operands sliced straight out of SBUF
                mt, m2t = direct
                gl = lambda: m2t[8 * jj:8 * jj + 8, :]
                gr = lambda c0, c1: mt[8 * jj:8 * jj + 8, c0:c1]
                tp = (8 * jj, 0)
            else:
                gl = lambda: lhs[:, jj, :]
                gr = lambda c0, c1: rhs[:, jj, c0:c1]
                tp = None
            ps = psM.tile([128, B], F32, tag="ps")
            nc.tensor.matmul(
                ps[:, 0:512], gl(), gr(0, 512),
                start=True, stop=False, skip_group_check=True,
                tile_position=tp)
            nc.tensor.matmul(
                ps[:, 0:512], nlh[:, jj, :], nrh[:, 0:512],
                start=False, stop=False, skip_group_check=True)
            nc.tensor.matmul(
                ps[:, 0:128], eye, eye,
                start=False, stop=True, skip_group_check=True)
            nc.tensor.matmul(
                ps[:, 512:1024], gl(), gr(512, 1024),
                start=True, stop=False, skip_group_check=True,
                tile_position=tp)
            nc.tensor.matmul(
                ps[:, 512:1024], nlh[:, jj, :], nrh[:, 512:1024],
                start=False, stop=True, skip_group_check=True)
            # LAM * bitcast(bits(sq) >> 1) ~= sqrt(sq)  (+-3%)
            psu = ps.bitcast(U32)
            nc.vector.tensor_scalar(
                out=psu, in0=psu, scalar1=1, scalar2=None,
                op0=OP.logical_shift_right)
            # e = exp(-sqrt(sq)), accumulated over b into feats[:, j]
            nc.scalar.activation(
                ps, ps, AF.Exp, scale=-LAM,
                accum_out=feats[:, j:j + 1])

        # prologue: chunks 0 and 1 fully prepped while input DMAs land;
        # steady state interleaves chunk ch+2's prep into chunk ch's loop.
        JORDER = list(range(JPC))
        mm0 = prep_a(0)
        nb = prep_b(0, mm0)
        gram = prep_c(0)
        mm1 = prep_a(1)
        nb_nxt = prep_b(1, mm1)
        gram_nxt = prep_c(1)
        for ch in range(NCH):
            nn_mm = None
            for step, jj in enumerate(JORDER):
                jwork(ch, gram, nb, jj)
                if ch + 2 < NCH:
                    if step == 0:
                        nn_mm = prep_a(ch + 2)
                    elif step == 2:
                        nb_nn = prep_b(ch + 2, nn_mm[0])
                    elif step == 4:
                        gram_nn = prep_c(ch + 2)
            if ch + 1 < NCH:
                gram, nb = gram_nxt, nb_nxt
            if ch + 2 < NCH:
                gram_nxt, nb_nxt = gram_nn, nb_nn
            # off-diag clamp floor (B-1)*exp(-10); diag contributes exactly 1
            csl = slice(ch * JPC, (ch + 1) * JPC)
            nc.gpsimd.tensor_scalar_add(feats[:, csl], feats[:, csl],
                                        1.0 + (B - 1) * C_CLAMP)
            nc.sync.dma_start(out=FEd.ap()[:, csl], in_=feats[:, csl])

    nc.finalize()
    return nc


def _get_program():
    if "nc" not in _PROG:
        _PROG["nc"] = _build_program()
    return _PROG["nc"]


def _bf16(a):
    import ml_dtypes
    return np.asarray(a, dtype=ml_dtypes.bfloat16)


def _host_consts():
    bd = np.zeros((128, JPC), dtype=np.float32)
    for p in range(128):
        bd[p, p // 8] = 1.0
    eye = _bf16(np.eye(128, dtype=np.float32) * DIAG_SQ)
    ones = np.ones((1, B), dtype=np.float32)
    oh = np.zeros((JPC, JPC, ROWS), dtype=np.float32)
    for jj in range(JPC):
        oh[jj, jj, :] = 1.0
    return bd, eye, ones, oh.reshape(JPC, JPC * ROWS)


def kernel(x: np.ndarray, T: np.ndarray) -> np.ndarray:
    from concourse.bass_utils import run_bass_kernel_spmd

    x = np.ascontiguousarray(np.asarray(x, dtype=np.float32))
    T = np.ascontiguousarray(np.asarray(T, dtype=np.float32))
    assert x.shape == (B, IN) and T.shape == (IN, J, K)

    nc = _get_program()
    t2 = _bf16(np.ascontiguousarray(T.reshape(IN, JK)))
    bd, eye, ones, oh = _host_consts()

    in_maps = []
    for c in range(NCORES):
        xr = np.roll(x, -c * ROWS, axis=0)            # local rows -> cols 0:128
        in_maps.append({
            "xTr": _bf16(np.ascontiguousarray(xr.T)),
            "T2": t2,
            "BD": bd,
            "EYE": eye,
            "ONESW": ones,
            "OH": oh,
        })

    res = run_bass_kernel_spmd(nc, in_maps, list(range(NCORES)))
    feats = np.concatenate([res.results[c]["FEATS"] for c in range(NCORES)], axis=0)
    return np.concatenate([x, feats.astype(np.float32)], axis=1)
